# revision 14
# baseline (speedup 1.0000x reference)
"""Trainium2 Bass kernel for ActorNetworkOriginal (GNN message passing).

Strategy (8-core SPMD, data-parallel over destination nodes):
  - Host: add self-loops, compute GCN norm coefficients, assign nodes to
    128-dst tiles with per-core degree balancing, pack per-edge compact
    tables (src gather indices, dst_local), and pre-gather raw node
    features per edge with the full edge norm folded in (G'' streams),
    so layer 1 needs no device-side gather and its scatter matrices are
    pure 0/1.
  - Device, per core:
      0/1 one-hot scatter matrices are GENERATED ON DEVICE (one
      broadcast tensor_tensor is_equal per dst tile) instead of being
      streamed from HBM -- removes ~36MB/core of DMA traffic.
      Layer 1: per dst tile, accumulate S[k,dst] with chunk matmuls
      (lhsT = norm-folded G'' chunk, rhs = generated 0/1 one-hot), lift
      with W1' + relu; h1 @ W2 for own rows is scaled by dis[node]
      (the src half of the GCN norm) and exchanged with a bf16
      AllGather; xw2 is interleaved with L1 tiles so the collective
      starts right as L1 ends.
      Layer 2: per-edge rows are fetched with SWDGE dma_gather on 4
      queues; descriptor generation for the first 6 dst tiles happens
      EARLY (prepare_only into per-tile buffers, triggered the moment
      the AllGather lands) and the remaining tiles gather directly in
      pairs, descriptor gen pipelined against the scatter.  The scatter
      runs TRANSPOSED (one-hot as lhsT -> out[dst, D]) so the dst half
      of the norm is a per-partition broadcast multiply; tiles are
      transposed back on the PE.  Graph pooling is one tiny static
      one-hot matmul per tile.  The virtual-node net runs inside the
      p-net AllGather window.  cur_v selection, summed skip connections
      and the 3-layer MLP close it out.
"""

import numpy as np
import ml_dtypes

import concourse.bass as bass
import concourse.tile as tile
from concourse import bacc, mybir
from concourse.bass_utils import run_bass_kernel_spmd
from concourse.masks import make_identity

F32 = mybir.dt.float32
BF16 = mybir.dt.bfloat16
I16 = mybir.dt.int16
OP = mybir.AluOpType
AX = mybir.AxisListType
NPBF = ml_dtypes.bfloat16

B, NPG, NVG = 64, 500, 50          # graphs, phys/virt nodes per graph
DPF, DVF, D = 16, 8, 128           # feature dims
NC = 8                             # cores
NP, NV = B * NPG, B * NVG          # 32000, 3200 total nodes
NPC, NVC = NP // NC, NV // NC      # 4000, 400 own nodes per core
GPC = B // NC                      # 8 graphs per core
NPT = (NPC + 127) // 128           # 32 p dst tiles / core
NVT = (NVC + 127) // 128           # 4 v dst tiles / core
PPAD = NPT * 128                   # 4096
VPAD = NVT * 128                   # 512
KP, KV = DPF + 1, DVF + 1          # ext feature dims (with bias row)
NQ = 4                             # SWDGE queues
NPREP = 6                          # dst tiles gathered via early preps

LAST_EXEC_NS = None
TRACE = False


# ----------------------------------------------------------------- host prep

def _prep_edges(edge_index, n_nodes, npc, ntiles):
    """Self-loops + norm; edges keyed by (core, dst tile); per-tile-slot
    padding to a core-independent chunk count."""
    src = np.asarray(edge_index[0], dtype=np.int64)
    dst = np.asarray(edge_index[1], dtype=np.int64)
    loops = np.arange(n_nodes, dtype=np.int64)
    src = np.concatenate([src, loops])
    dst = np.concatenate([dst, loops])
    deg = np.bincount(dst, minlength=n_nodes).astype(np.float32)
    dis = 1.0 / np.sqrt(deg)
    norm = (dis[src] * dis[dst]).astype(np.float32)

    core = dst // npc
    rem = dst % npc
    tid = rem // 128
    dloc = rem % 128
    key = core * ntiles + tid
    order = np.argsort(key, kind="stable")
    src, dloc, norm, key = src[order], dloc[order], norm[order], key[order]
    counts = np.bincount(key, minlength=NC * ntiles).reshape(NC, ntiles)
    cpt = np.maximum(1, -(-counts.max(axis=0) // 128)).astype(int)
    csum = np.concatenate([[0], np.cumsum(counts.ravel())])
    epc = int(cpt.sum()) * 128
    src_p = np.zeros((NC, epc), np.int64)
    dl_p = np.full((NC, epc), 200, np.int64)   # pads miss the 0..127 iota
    nm_p = np.zeros((NC, epc), np.float32)
    for c in range(NC):
        off = 0
        for t in range(ntiles):
            k = c * ntiles + t
            a, b = int(csum[k]), int(csum[k + 1])
            n = b - a
            src_p[c, off:off + n] = src[a:b]
            dl_p[c, off:off + n] = dloc[a:b]
            nm_p[c, off:off + n] = norm[a:b]
            off += int(cpt[t]) * 128
    return src_p, dl_p, nm_p, cpt


def _prep_edges_balanced(edge_index, n_nodes, npc, ntiles, tpad):
    """p-net variant: per-core degree-balanced node->tile assignment."""
    npt = npc // ntiles                    # nodes per tile (125)
    src = np.asarray(edge_index[0], dtype=np.int64)
    dst = np.asarray(edge_index[1], dtype=np.int64)
    loops = np.arange(n_nodes, dtype=np.int64)
    src = np.concatenate([src, loops])
    dst = np.concatenate([dst, loops])
    deg = np.bincount(dst, minlength=n_nodes).astype(np.float32)
    dis = 1.0 / np.sqrt(deg)
    norm = (dis[src] * dis[dst]).astype(np.float32)

    tile_of = np.empty(n_nodes, np.int64)
    pos_of = np.empty(n_nodes, np.int64)
    for c in range(NC):
        lo = c * npc
        nodes = np.arange(lo, lo + npc)
        order = np.argsort(-deg[nodes], kind="stable")
        loads = np.zeros(ntiles)
        fill = np.zeros(ntiles, np.int64)
        for nd in nodes[order]:
            cand = np.where(fill < npt)[0]
            t = cand[np.argmin(loads[cand])]
            tile_of[nd] = t
            pos_of[nd] = fill[t]
            loads[t] += deg[nd]
            fill[t] += 1
    tid = tile_of[dst]
    dloc = pos_of[dst]
    key = (dst // npc) * ntiles + tid
    order = np.argsort(key, kind="stable")
    src, dloc, norm, key = src[order], dloc[order], norm[order], key[order]
    counts = np.bincount(key, minlength=NC * ntiles).reshape(NC, ntiles)
    cpt = np.maximum(1, -(-counts.max(axis=0) // 128)).astype(int)
    csum = np.concatenate([[0], np.cumsum(counts.ravel())])
    epc = int(cpt.sum()) * 128
    src_p = np.zeros((NC, epc), np.int64)
    dl_p = np.full((NC, epc), 200, np.int64)   # pads miss the 0..127 iota
    nm_p = np.zeros((NC, epc), np.float32)
    for c in range(NC):
        off = 0
        for t in range(ntiles):
            k = c * ntiles + t
            a, b = int(csum[k]), int(csum[k + 1])
            n = b - a
            src_p[c, off:off + n] = src[a:b]
            dl_p[c, off:off + n] = dloc[a:b]
            nm_p[c, off:off + n] = norm[a:b]
            off += int(cpt[t]) * 128
    perm = np.full((NC, tpad), -1, np.int64)
    for nd in range(n_nodes):
        c = nd // npc
        perm[c, tile_of[nd] * 128 + pos_of[nd]] = nd
    return src_p, dl_p, nm_p, cpt, perm, dis


def _pack_idx(src):
    """[NC, E] node ids -> [NC, 128, E//16] int16 (16-partition wrap,
    replicated to all 8 partition groups)."""
    n = src.shape[1]
    w = src.astype(np.int16).reshape(NC, n // 16, 16).transpose(0, 2, 1)
    return np.ascontiguousarray(np.tile(w, (1, 8, 1)))


def _pack_lane(vals, dtype):
    """[NC, E] per-edge values -> [NC, 128, E//128] lane-major tables."""
    n = vals.shape[1]
    w = vals.reshape(NC, n // 128, 128).transpose(0, 2, 1)
    return np.ascontiguousarray(w.astype(dtype))


def _build_G(src, nm, xT):
    """[NC, E] src ids + per-edge norm + [k, N] f32 ext features ->
    [NC, 128, nch, k] bf16 norm-folded pre-gathered chunks (lhsT)."""
    g = xT[:, src] * nm[None, :, :]              # [k, NC, E]
    g = np.transpose(g, (1, 2, 0))               # [NC, E, k]
    nch = g.shape[1] // 128
    k = g.shape[2]
    return np.ascontiguousarray(
        g.reshape(NC, nch, 128, k).transpose(0, 2, 1, 3).astype(NPBF))


# ------------------------------------------------------------- device build

def _build(cpt_p, cpt_v):
    nc = bacc.Bacc("TRN2", target_bir_lowering=False, debug=False,
                   num_devices=NC, num_swdge_queues=NQ)

    chp = int(cpt_p.sum())          # total p chunks per core
    chv = int(cpt_v.sum())
    cmax = int(cpt_p.max())
    coff_p = np.concatenate([[0], np.cumsum(cpt_p)]).astype(int)
    coff_v = np.concatenate([[0], np.cumsum(cpt_v)]).astype(int)
    pairs = [(t, t + 1) for t in range(0, NPT, 2)]

    def din(name, shape, dtype=F32):
        return nc.dram_tensor(name, shape, dtype, kind="ExternalInput")

    # inputs
    pxT_o = din("p_xT_own", [KP, PPAD])
    vxT_o = din("v_xT_own", [KV, NVC])
    wf1p = din("wf1p", [KP, D], BF16)
    wep = din("wep", [KP, D])
    w2p = din("w2p", [D, D], BF16)
    b1p = din("b1p", [D, 1])
    b2p = din("b2p", [D, 1])
    wf1v = din("wf1v", [KV, D], BF16)
    wev = din("wev", [KV, D])
    w2v = din("w2v", [D, D], BF16)
    b1v = din("b1v", [D, 1])
    b2v = din("b2v", [D, 1])
    w1a = din("w1a", [D, D], BF16)
    w1b = din("w1b", [D, D], BF16)
    b1a = din("b1a", [D, 1])
    b1b = din("b1b", [D, 1])
    w2a = din("w2a", [D, D], BF16)
    w2b = din("w2b", [D, D], BF16)
    b2m = din("b2m", [D, 1])
    w3 = din("w3", [D, 1], BF16)
    b3 = din("b3", [1, 1])
    sel = din("sel", [128, NVT * GPC])
    gt = din("gt", [GPC, PPAD])
    b2r = din("b2r", [GPC, D])
    gp_d = din("Gp", [128, chp, KP], BF16)
    gv_d = din("Gv", [128, chv, KV], BF16)
    idxp2 = din("idxs_p2", [128, chp * 8], I16)
    idxv1 = din("idxs_v1", [128, chv * 8], I16)
    dstp_d = din("dstp", [128, chp])
    oh1p_d = din("oh1p", [128, chp * 128], BF16)
    dstv_d = din("dstv", [128, chv])
    nrmv_d = din("nrmv", [128, chv])
    diso_d = din("diso", [128, NPT])
    gtt_d = din("gtt", [128, NPT * GPC])
    iota_d = din("iota", [128, 128])

    out_d = nc.dram_tensor("out", [1, PPAD], F32, kind="ExternalOutput")

    # internal DRAM (bf16 exchange)
    ccip = nc.dram_tensor("ccip", [PPAD, D], BF16, kind="Internal")
    ccop = nc.dram_tensor("ccop", [NC * PPAD, D], BF16, kind="Internal",
                          addr_space="Shared")
    cciv = nc.dram_tensor("cciv", [NVC, D], BF16, kind="Internal")
    ccov = nc.dram_tensor("ccov", [NC * NVC, D], BF16, kind="Internal",
                          addr_space="Shared")

    with tile.TileContext(nc) as tc:
        with (
            tc.tile_pool(name="meta", bufs=1) as meta,
            tc.tile_pool(name="bigp", bufs=3) as bigp,
            tc.tile_pool(name="vsml", bufs=1) as vsml,
            tc.tile_pool(name="stp", bufs=3) as stp,
            tc.tile_pool(name="ohp", bufs=1) as ohp,
            tc.tile_pool(name="gat", bufs=1) as gat,
            tc.tile_pool(name="psA", bufs=3, space="PSUM") as psA,
            tc.tile_pool(name="psB", bufs=4, space="PSUM") as psB,
            tc.tile_pool(name="psC", bufs=1, space="PSUM") as psC,
        ):
            def load(dram, eng=None, tag=None):
                t = meta.tile(list(dram.shape), dram.dtype,
                              tag=tag or dram.name)
                (eng or nc.sync).dma_start(out=t[:], in_=dram[:])
                return t

            iota_sb = load(iota_d)
            dstp_sb = load(dstp_d)
            gp_sb = load(gp_d)
            wf1p_sb = load(wf1p)
            b1p_sb = load(b1p)
            wep_sb = load(wep)
            diso_sb = load(diso_d)
            idxp2_sb = load(idxp2, nc.scalar)
            idxv1_sb = load(idxv1, nc.scalar)
            dstv_sb = load(dstv_d, nc.scalar)
            nrmv_sb = load(nrmv_d, nc.scalar)
            gv_sb = load(gv_d, nc.scalar)
            gtt_sb = load(gtt_d, nc.scalar)
            b2r_sb = load(b2r, nc.scalar)
            w2p_sb = load(w2p, nc.scalar); b2p_sb = load(b2p, nc.scalar)
            wf1v_sb = load(wf1v, nc.scalar); wev_sb = load(wev, nc.scalar)
            w2v_sb = load(w2v, nc.scalar)
            b1v_sb = load(b1v, nc.scalar); b2v_sb = load(b2v, nc.scalar)
            w1a_sb = load(w1a, nc.scalar); w1b_sb = load(w1b, nc.scalar)
            b1a_sb = load(b1a, nc.scalar); b1b_sb = load(b1b, nc.scalar)
            w2a_sb = load(w2a, nc.scalar); w2b_sb = load(w2b, nc.scalar)
            b2m_sb = load(b2m, nc.scalar)
            w3_sb = load(w3, nc.scalar); b3_sb = load(b3, nc.scalar)
            sel_sb = load(sel, nc.scalar)
            gt_sb = load(gt, nc.scalar)

            ident = meta.tile([128, 128], F32, tag="ident")
            make_identity(nc, ident[:])

            initp = meta.tile([128, PPAD], BF16, tag="initp")
            initv = meta.tile([128, VPAD], F32, tag="initv")
            nc.vector.memset(initv[:], 0.0)

            pxTo_sb = meta.tile([KP, PPAD], F32, tag="pxTo")
            nc.sync.dma_start(out=pxTo_sb[:], in_=pxT_o[:])
            vxTo_sb = meta.tile([KV, NVC], F32, tag="vxTo")
            nc.sync.dma_start(out=vxTo_sb[:], in_=vxT_o[:])

            # ---- one-hot generation: one broadcast TT per dst tile
            def gen_oh01(ohb, c0, n_ch, dst_sb, eng):
                eng.tensor_tensor(
                    out=ohb[:, 0:n_ch, :],
                    in0=iota_sb[:].unsqueeze(1)
                        .broadcast_to([128, n_ch, 128]),
                    in1=dst_sb[:, c0:c0 + n_ch].unsqueeze(2)
                        .broadcast_to([128, n_ch, 128]),
                    op=OP.is_equal)

            # ---- initT own (feat x own nodes, tile layout)
            for j in range(PPAD // 512):
                acc = psB.tile([128, 512], F32, tag="b512", space="PSUM")
                nc.tensor.matmul(acc[:],
                                 wep_sb[:],
                                 pxTo_sb[:, j * 512:(j + 1) * 512],
                                 start=True, stop=True)
                nc.vector.tensor_copy(out=initp[:, j * 512:(j + 1) * 512],
                                      in_=acc[:])
            acc = psB.tile([128, 512], F32, tag="b512", space="PSUM")
            nc.tensor.matmul(acc[:, :NVC], wev_sb[:], vxTo_sb[:],
                             start=True, stop=True)
            nc.vector.tensor_copy(out=initv[:, :NVC], in_=acc[:, :NVC])

            # ---- layer 1 (p then v), xw2 fused per 4 tiles
            def l1_tile(t, g_sb, kd, cpt, coff, dst_sb, h_sb, b_sb,
                        wf_sb, eng, oh_dram=None):
                n_ch = int(cpt[t])
                ohb = ohp.tile([128, cmax, 128], BF16, tag=f"oh{t % 3}")
                if oh_dram is not None:
                    nc.sync.dma_start(
                        out=ohb[:, 0:n_ch, :],
                        in_=oh_dram[:, int(coff[t]) * 128:
                                    (int(coff[t]) + n_ch) * 128]
                        .rearrange("p (a b) -> p a b", a=n_ch))
                else:
                    gen_oh01(ohb, int(coff[t]), n_ch, dst_sb, eng)
                s_ps = psA.tile([128, 136], F32, tag="pacc", space="PSUM")
                for j in range(n_ch):
                    nc.tensor.matmul(
                        s_ps[:kd, :128],
                        g_sb[:, int(coff[t]) + j, :],
                        ohb[:, j, :],
                        start=(j == 0), stop=(j == n_ch - 1))
                s_sb = stp.tile([kd, 128], BF16, tag=f"ssb{kd}")
                nc.vector.tensor_copy(out=s_sb[:], in_=s_ps[:kd, :128])
                acc = psA.tile([128, 136], F32, tag="pacc", space="PSUM")
                nc.tensor.matmul(acc[:, :128], wf_sb[:], s_sb[:],
                                 start=True, stop=True)
                nc.scalar.activation(
                    out=h_sb[:, t * 128:(t + 1) * 128], in_=acc[:, :128],
                    func=mybir.ActivationFunctionType.Relu,
                    bias=b_sb[:, 0:1])

            def xw2_block(h_sb, w_sb, cci, off, nb, nrows, dis_sb):
                acc = psB.tile([128, nb * 128], F32, tag="b512",
                               space="PSUM")
                for j in range(nb):
                    i = off + j
                    nc.tensor.matmul(
                        acc[:, j * 128:(j + 1) * 128],
                        h_sb[:, i * 128:(i + 1) * 128],
                        w_sb[:], start=True, stop=True)
                stg = stp.tile([128, nb, 128], BF16, tag="stg")
                if dis_sb is None:
                    nc.vector.tensor_copy(out=stg[:], in_=acc[:])
                else:
                    for j in range(nb):
                        nc.vector.tensor_tensor(
                            out=stg[:, j, :],
                            in0=acc[:, j * 128:(j + 1) * 128],
                            in1=dis_sb[:, off + j:off + j + 1]
                                .broadcast_to([128, 128]),
                            op=OP.mult)
                nfull = min(nb, (nrows - off * 128) // 128)
                if nfull > 0:
                    nc.sync.dma_start(
                        out=cci[off * 128:(off + nfull) * 128, :]
                        .rearrange("(t p) f -> p t f", p=128),
                        in_=stg[:, :nfull, :])
                rem = nrows - (off + nfull) * 128
                if 0 < rem < 128 and nfull < nb:
                    nc.sync.dma_start(
                        out=cci[(off + nfull) * 128:nrows, :],
                        in_=stg[:rem, nfull, :])

            # ---- v layer 1 + exchange first: AG-v and the v gather
            #      hide entirely under p layer 1 / AG-p
            h1v = vsml.tile([128, VPAD], BF16, tag="h1v")
            for t in range(NVT):
                l1_tile(t, gv_sb, KV, cpt_v, coff_v, dstv_sb, h1v,
                        b1v_sb, wf1v_sb, nc.vector)
            xw2_block(h1v, w2v_sb, cciv, 0, 4, NVC, None)
            nc.gpsimd.collective_compute(
                "AllGather", OP.bypass,
                ins=[cciv[:]], outs=[ccov[:]],
                replica_groups=[list(range(NC))])
            vgb = gat.tile([128, chv, D], BF16, tag="gbv")
            nc.gpsimd.dma_gather(
                vgb[:], ccov[:], idxv1_sb[:], chv * 128, chv * 128, D,
                single_packet=False, queue_num=2)

            h1p = bigp.tile([128, PPAD], BF16, tag="bigh")
            for t in range(NPT):
                l1_tile(t, gp_sb, KP, cpt_p, coff_p, dstp_sb, h1p,
                        b1p_sb, wf1p_sb, nc.vector, oh_dram=oh1p_d)
                if t % 4 == 3:
                    xw2_block(h1p, w2p_sb, ccip, t - 3, 4, PPAD, diso_sb)

            nc.gpsimd.collective_compute(
                "AllGather", OP.bypass,
                ins=[ccip[:]], outs=[ccop[:]],
                replica_groups=[list(range(NC))])

            # ---- p layer 2: direct gathers in pairs, scatter chasing
            h2p = bigp.tile([128, PPAD], BF16, tag="bigh")
            gsps = psC.tile([GPC, D], F32, tag="gs", space="PSUM")

            def scatter_ptile(t, m, mbase):
                n_ch = int(cpt_p[t])
                ohb = ohp.tile([128, cmax, 128], BF16, tag=f"oh{t % 3}")
                gen_oh01(ohb, int(coff_p[t]), n_ch, dstp_sb, nc.vector)
                acc = psA.tile([128, 136], F32, tag="pacc", space="PSUM")
                for j in range(n_ch):
                    nc.tensor.matmul(
                        acc[:, :128], ohb[:, j, :], m[:, mbase + j, :],
                        start=(j == 0), stop=(j == n_ch - 1))
                h2T = stp.tile([128, 128], F32, tag=f"h2T{t % 3}")
                nc.vector.tensor_tensor(
                    out=h2T[:], in0=acc[:, :128],
                    in1=diso_sb[:, t:t + 1].broadcast_to([128, 128]),
                    op=OP.mult)
                trp = psA.tile([128, 136], F32, tag="pacc", space="PSUM")
                nc.tensor.transpose(trp[:, :128], h2T[:], ident[:])
                nc.scalar.activation(
                    out=h2p[:, t * 128:(t + 1) * 128], in_=trp[:, :128],
                    func=mybir.ActivationFunctionType.Identity,
                    bias=b2p_sb[:, 0:1])
                nc.tensor.matmul(gsps[:], gtt_sb[:, t * GPC:(t + 1) * GPC],
                                 h2T[:], start=(t == 0), stop=(t == NPT - 1))

            dbufs = {}

            def direct_pair(i):
                t0, t1 = pairs[i]
                n_ch = int(cpt_p[t0] + cpt_p[t1])
                buf = gat.tile([128, 2 * cmax, D], BF16, tag=f"gd{i % 6}")
                dbufs[i] = buf
                e0 = int(coff_p[t0]) * 128
                n = n_ch * 128
                nc.gpsimd.dma_gather(
                    buf[:, :n_ch, :], ccop[:],
                    idxp2_sb[:, e0 // 16:(e0 + n) // 16],
                    n, n, D, single_packet=False, queue_num=i % NQ)

            direct_pair(0)
            direct_pair(1)
            direct_pair(2)
            h2v = vsml.tile([128, VPAD], F32, tag="h2v")
            for i in range(len(pairs)):
                if i + 3 < len(pairs):
                    direct_pair(i + 3)
                t0, t1 = pairs[i]
                buf = dbufs.pop(i)
                scatter_ptile(t0, buf, 0)
                scatter_ptile(t1, buf, int(cpt_p[t0]))
                if i == 0:
                    # v layer 2 (0/1 one-hot then per-lane norm fold)
                    for t in range(NVT):
                        n_ch = int(cpt_v[t])
                        ohb = ohp.tile([128, cmax, 128], BF16,
                                       tag=f"oh{t % 3}")
                        gen_oh01(ohb, int(coff_v[t]), n_ch, dstv_sb,
                                 nc.vector)
                        nc.vector.tensor_tensor(
                            out=ohb[:, 0:n_ch, :], in0=ohb[:, 0:n_ch, :],
                            in1=nrmv_sb[:, int(coff_v[t]):
                                        int(coff_v[t]) + n_ch]
                                .unsqueeze(2)
                                .broadcast_to([128, n_ch, 128]),
                            op=OP.mult)
                        acc = psA.tile([128, 136], F32, tag="pacc",
                                       space="PSUM")
                        for j in range(n_ch):
                            nc.tensor.matmul(
                                acc[:, :128],
                                vgb[:, int(coff_v[t]) + j, :],
                                ohb[:, j, :],
                                start=(j == 0), stop=(j == n_ch - 1))
                        nc.scalar.activation(
                            out=h2v[:, t * 128:(t + 1) * 128],
                            in_=acc[:, :128],
                            func=mybir.ActivationFunctionType.Identity,
                            bias=b2v_sb[:, 0:1])

            # ---- v pools / v summed / cur_v
            gv = vsml.tile([128, GPC], F32, tag="gv")
            for g in range(GPC):
                nc.vector.reduce_sum(out=gv[:, g:g + 1],
                                     in_=h2v[:, g * NVG:(g + 1) * NVG],
                                     axis=AX.X)
            gvs = vsml.tile([128, GPC], F32, tag="gvs")
            nc.vector.tensor_scalar(out=gvs[:], in0=gv[:],
                                    scalar1=1.0 / NVG, scalar2=None,
                                    op0=OP.mult)
            vsum = vsml.tile([128, VPAD], F32, tag="vsum")
            nc.vector.tensor_tensor(out=vsum[:], in0=h2v[:], in1=initv[:],
                                    op=OP.add)
            for g in range(GPC):
                nc.scalar.activation(
                    out=vsum[:, g * NVG:(g + 1) * NVG],
                    in_=vsum[:, g * NVG:(g + 1) * NVG],
                    func=mybir.ActivationFunctionType.Identity,
                    bias=gvs[:, g:g + 1])
            curv_ps = psA.tile([128, GPC], F32, tag="pacc", space="PSUM")
            for k in range(NVT):
                trp = psA.tile([128, 128], F32, tag="pacc", space="PSUM")
                nc.tensor.transpose(trp[:], vsum[:, k * 128:(k + 1) * 128],
                                    ident[:])
                vs_rm = vsml.tile([128, 128], F32, tag="vsrm")
                nc.vector.tensor_copy(out=vs_rm[:], in_=trp[:])
                nc.tensor.matmul(curv_ps[:], vs_rm[:],
                                 sel_sb[:, k * GPC:(k + 1) * GPC],
                                 start=(k == 0), stop=(k == NVT - 1))
            curv_sb = vsml.tile([128, GPC], F32, tag="curvsb")
            nc.vector.tensor_copy(out=curv_sb[:], in_=curv_ps[:])

            # gcT[g, d] = gsum/NPG + b2 + curv  (graph mean of h2 + cur_v)
            curvT_ps = psA.tile([128, 128], F32, tag="pacc", space="PSUM")
            nc.tensor.transpose(curvT_ps[:GPC, :], curv_sb[:], ident[:])
            gcT = vsml.tile([GPC, 128], F32, tag="gcT")
            nc.vector.tensor_scalar(out=gcT[:], in0=gsps[:],
                                    scalar1=1.0 / NPG, scalar2=None,
                                    op0=OP.mult)
            nc.vector.tensor_tensor(out=gcT[:], in0=gcT[:],
                                    in1=b2r_sb[:], op=OP.add)
            nc.vector.tensor_tensor(out=gcT[:], in0=gcT[:],
                                    in1=curvT_ps[:GPC, :], op=OP.add)

            state = bigp.tile([128, PPAD], BF16, tag="bigh")
            nc.vector.tensor_tensor(out=state[:], in0=h2p[:],
                                    in1=initp[:], op=OP.add)
            for n in range(PPAD // 512):
                sl = slice(n * 512, (n + 1) * 512)
                gcx = psB.tile([128, 512], F32, tag="b512", space="PSUM")
                nc.tensor.matmul(gcx[:], gcT[:], gt_sb[:, sl],
                                 start=True, stop=True)
                nc.vector.tensor_tensor(out=state[:, sl],
                                        in0=state[:, sl],
                                        in1=gcx[:], op=OP.add)

            # ---- MLP
            mh1a = bigp.tile([128, PPAD], BF16, tag="bigh")
            mh1b = bigp.tile([128, PPAD], BF16, tag="bigh")
            for (w_sb, b_sb, mh) in ((w1a_sb, b1a_sb, mh1a),
                                     (w1b_sb, b1b_sb, mh1b)):
                for n in range(PPAD // 512):
                    sl = slice(n * 512, (n + 1) * 512)
                    acc = psB.tile([128, 512], F32, tag="b512",
                                   space="PSUM")
                    nc.tensor.matmul(acc[:], w_sb[:], state[:, sl],
                                     start=True, stop=True)
                    nc.scalar.activation(
                        out=mh[:, sl], in_=acc[:],
                        func=mybir.ActivationFunctionType.Relu,
                        bias=b_sb[:, 0:1])
            mh2 = bigp.tile([128, PPAD], BF16, tag="bigh")
            for n in range(PPAD // 512):
                sl = slice(n * 512, (n + 1) * 512)
                acc = psB.tile([128, 512], F32, tag="b512", space="PSUM")
                nc.tensor.matmul(acc[:], w2a_sb[:], mh1a[:, sl],
                                 start=True, stop=False)
                nc.tensor.matmul(acc[:], w2b_sb[:], mh1b[:, sl],
                                 start=False, stop=True)
                nc.scalar.activation(
                    out=mh2[:, sl], in_=acc[:],
                    func=mybir.ActivationFunctionType.Relu,
                    bias=b2m_sb[:, 0:1])
            for n in range(PPAD // 512):
                sl = slice(n * 512, (n + 1) * 512)
                accl = psA.tile([1, 512], F32, tag="pacc", space="PSUM")
                nc.tensor.matmul(accl[:], w3_sb[:], mh2[:, sl],
                                 start=True, stop=True)
                lgc = stp.tile([1, 512], F32, tag="lgc")
                nc.vector.tensor_scalar(
                    out=lgc[0:1, :], in0=accl[:], scalar1=b3_sb[0:1, 0:1],
                    scalar2=None, op0=OP.add)
                nc.sync.dma_start(out=out_d[0:1, sl], in_=lgc[0:1, :])

    nc.compile()
    return nc


# ------------------------------------------------------------------- kernel

def kernel(**inputs):
    global LAST_EXEC_NS
    f = lambda k: np.asarray(inputs[k], dtype=np.float32)

    # edge preprocessing
    sp, dlp, nmp, cpt_p, perm, disp = _prep_edges_balanced(
        np.asarray(inputs["p_edge_index"]), NP, NPC, NPT, PPAD)
    sv, dlv, nmv, cpt_v = _prep_edges(np.asarray(inputs["v_edge_index"]),
                                      NV, NVC, NVT)
    # L2 p rows live at permuted positions: node -> core*PPAD + tile*128+pos
    posmap = np.empty(NP, np.int64)          # node -> tile*128+pos
    for c in range(NC):
        valid = perm[c] >= 0
        posmap[perm[c][valid]] = np.nonzero(valid)[0]
    sp2 = (sp // NPC) * PPAD + posmap[sp]

    idxs_p2 = _pack_idx(sp2)
    idxs_v1 = _pack_idx(sv)
    dstp = _pack_lane(dlp, np.float32)
    dstv = _pack_lane(dlv, np.float32)
    nrmv = _pack_lane(nmv, np.float32)
    iota = np.ascontiguousarray(
        np.broadcast_to(np.arange(128, dtype=np.float32)[None, :],
                        (128, 128)))

    # weights
    p_x = f("p_x"); v_x = f("v_x")
    wep = np.vstack([f("p_lin_w"), f("p_lin_b")[None, :]])
    wev = np.vstack([f("v_lin_w"), f("v_lin_b")[None, :]])
    wf1p = wep @ f("p_gcn_w1")
    wf1v = wev @ f("v_gcn_w1")
    pxT = np.vstack([p_x.T, np.ones((1, NP), np.float32)])
    vxT = np.vstack([v_x.T, np.ones((1, NV), np.float32)])
    act = np.asarray(inputs["high_level_action"]).astype(np.int64)

    # pre-gathered per-edge features with edge norm folded in
    Gp = _build_G(sp, nmp, pxT)
    Gv = _build_G(sv, nmv, vxT)

    base = {
        "wf1p": wf1p.astype(NPBF), "wep": wep,
        "w2p": f("p_gcn_w2").astype(NPBF),
        "b1p": f("p_gcn_b1")[:, None], "b2p": f("p_gcn_b2")[:, None],
        "wf1v": wf1v.astype(NPBF), "wev": wev,
        "w2v": f("v_gcn_w2").astype(NPBF),
        "b1v": f("v_gcn_b1")[:, None], "b2v": f("v_gcn_b2")[:, None],
        "w1a": f("low_w1")[:, :D].astype(NPBF),
        "w1b": f("low_w1")[:, D:].astype(NPBF),
        "b1a": f("low_b1")[:D, None], "b1b": f("low_b1")[D:, None],
        "w2a": f("low_w2")[:D, :].astype(NPBF),
        "w2b": f("low_w2")[D:, :].astype(NPBF),
        "b2m": f("low_b2")[:, None],
        "w3": f("low_w3").astype(NPBF), "b3": f("low_b3")[:, None],
        "b2r": np.broadcast_to(f("p_gcn_b2")[None, :], (GPC, D)),
        "iota": iota,
    }
    base = {k: (np.ascontiguousarray(v) if v.dtype == NPBF
                else np.ascontiguousarray(v, dtype=np.float32))
            for k, v in base.items()}

    pgraph = np.asarray(inputs["p_batch"]).astype(np.int64)

    in_maps = []
    for c in range(NC):
        selm = np.zeros((128, NVT * GPC), np.float32)
        for g in range(GPC):
            r = g * NVG + int(act[c * GPC + g])
            selm[r % 128, (r // 128) * GPC + g] = 1.0
        pxo = np.zeros((KP, PPAD), np.float32)
        gtm = np.zeros((GPC, PPAD), np.float32)
        disown = np.zeros(PPAD, np.float32)
        gttm = np.zeros((128, NPT * GPC), np.float32)
        valid = perm[c] >= 0
        cols = np.nonzero(valid)[0]
        nodes = perm[c][valid]
        pxo[:, cols] = pxT[:, nodes]
        gtm[pgraph[nodes] % GPC, cols] = 1.0
        disown[cols] = disp[nodes]
        gttm[cols % 128, (cols // 128) * GPC + pgraph[nodes] % GPC] = 1.0
        m = dict(base)
        m["p_xT_own"] = pxo
        m["v_xT_own"] = np.ascontiguousarray(
            vxT[:, c * NVC:(c + 1) * NVC])
        m["sel"] = selm
        m["gt"] = gtm
        m["diso"] = np.ascontiguousarray(
            disown.reshape(NPT, 128).T.astype(np.float32))
        m["gtt"] = np.ascontiguousarray(gttm)
        oh1 = np.zeros((128, dlp.shape[1] // 128, 128), NPBF)
        e = np.arange(dlp.shape[1])
        real = dlp[c] < 128
        oh1[e[real] % 128, e[real] // 128, dlp[c][real]] = 1.0
        m["oh1p"] = np.ascontiguousarray(oh1.reshape(128, -1))
        m["idxs_p2"] = idxs_p2[c]
        m["idxs_v1"] = idxs_v1[c]
        m["dstp"] = dstp[c]; m["dstv"] = dstv[c]; m["nrmv"] = nrmv[c]
        m["Gp"] = Gp[c]; m["Gv"] = Gv[c]
        in_maps.append(m)

    nc = _build(cpt_p, cpt_v)
    res = run_bass_kernel_spmd(nc, in_maps, core_ids=list(range(NC)),
                               trace=TRACE)
    LAST_EXEC_NS = res.exec_time_ns
    out = np.empty((NC, NPC), np.float32)
    for c in range(NC):
        lgv = res.results[c]["out"][0]
        valid = perm[c] >= 0
        out[c][perm[c][valid] - c * NPC] = lgv[valid]
    return out.reshape(B, NPG).astype(np.float32)


# revision 15
# speedup vs baseline: 1.0486x; 1.0486x over previous
"""Trainium2 Bass kernel for ActorNetworkOriginal (GNN message passing).

Strategy (8-core SPMD, data-parallel over destination nodes):
  - Host: add self-loops, compute GCN norm coefficients, assign nodes to
    128-dst tiles with per-core degree balancing, pack per-edge compact
    tables (src gather indices, dst_local), and pre-gather raw node
    features per edge with the full edge norm folded in (G'' streams),
    so layer 1 needs no device-side gather and its scatter matrices are
    pure 0/1.
  - Device, per core:
      0/1 one-hot scatter matrices are GENERATED ON DEVICE (one
      broadcast tensor_tensor is_equal per dst tile) instead of being
      streamed from HBM -- removes ~36MB/core of DMA traffic.
      Layer 1: per dst tile, accumulate S[k,dst] with chunk matmuls
      (lhsT = norm-folded G'' chunk, rhs = generated 0/1 one-hot), lift
      with W1' + relu; h1 @ W2 for own rows is scaled by dis[node]
      (the src half of the GCN norm) and exchanged with a bf16
      AllGather; xw2 is interleaved with L1 tiles so the collective
      starts right as L1 ends.
      Layer 2: per-edge rows are fetched with SWDGE dma_gather on 4
      queues; descriptor generation for the first 6 dst tiles happens
      EARLY (prepare_only into per-tile buffers, triggered the moment
      the AllGather lands) and the remaining tiles gather directly in
      pairs, descriptor gen pipelined against the scatter.  The scatter
      runs TRANSPOSED (one-hot as lhsT -> out[dst, D]) so the dst half
      of the norm is a per-partition broadcast multiply; tiles are
      transposed back on the PE.  Graph pooling is one tiny static
      one-hot matmul per tile.  The virtual-node net runs inside the
      p-net AllGather window.  cur_v selection, summed skip connections
      and the 3-layer MLP close it out.
"""

import numpy as np
import ml_dtypes

import concourse.bass as bass
import concourse.tile as tile
from concourse import bacc, mybir
from concourse.bass_utils import run_bass_kernel_spmd
from concourse.masks import make_identity

F32 = mybir.dt.float32
BF16 = mybir.dt.bfloat16
I16 = mybir.dt.int16
OP = mybir.AluOpType
AX = mybir.AxisListType
NPBF = ml_dtypes.bfloat16

B, NPG, NVG = 64, 500, 50          # graphs, phys/virt nodes per graph
DPF, DVF, D = 16, 8, 128           # feature dims
NC = 8                             # cores
NP, NV = B * NPG, B * NVG          # 32000, 3200 total nodes
NPC, NVC = NP // NC, NV // NC      # 4000, 400 own nodes per core
GPC = B // NC                      # 8 graphs per core
NPT = (NPC + 127) // 128           # 32 p dst tiles / core
NVT = (NVC + 127) // 128           # 4 v dst tiles / core
PPAD = NPT * 128                   # 4096
VPAD = NVT * 128                   # 512
KP, KV = DPF + 1, DVF + 1          # ext feature dims (with bias row)
NQ = 4                             # SWDGE queues
NPREP = 6                          # dst tiles gathered via early preps

LAST_EXEC_NS = None
TRACE = False


# ----------------------------------------------------------------- host prep

def _prep_edges(edge_index, n_nodes, npc, ntiles):
    """Self-loops + norm; edges keyed by (core, dst tile); per-tile-slot
    padding to a core-independent chunk count."""
    src = np.asarray(edge_index[0], dtype=np.int64)
    dst = np.asarray(edge_index[1], dtype=np.int64)
    loops = np.arange(n_nodes, dtype=np.int64)
    src = np.concatenate([src, loops])
    dst = np.concatenate([dst, loops])
    deg = np.bincount(dst, minlength=n_nodes).astype(np.float32)
    dis = 1.0 / np.sqrt(deg)
    norm = (dis[src] * dis[dst]).astype(np.float32)

    core = dst // npc
    rem = dst % npc
    tid = rem // 128
    dloc = rem % 128
    key = core * ntiles + tid
    order = np.argsort(key, kind="stable")
    src, dloc, norm, key = src[order], dloc[order], norm[order], key[order]
    counts = np.bincount(key, minlength=NC * ntiles).reshape(NC, ntiles)
    cpt = np.maximum(1, -(-counts.max(axis=0) // 128)).astype(int)
    csum = np.concatenate([[0], np.cumsum(counts.ravel())])
    epc = int(cpt.sum()) * 128
    src_p = np.zeros((NC, epc), np.int64)
    dl_p = np.full((NC, epc), 200, np.int64)   # pads miss the 0..127 iota
    nm_p = np.zeros((NC, epc), np.float32)
    for c in range(NC):
        off = 0
        for t in range(ntiles):
            k = c * ntiles + t
            a, b = int(csum[k]), int(csum[k + 1])
            n = b - a
            src_p[c, off:off + n] = src[a:b]
            dl_p[c, off:off + n] = dloc[a:b]
            nm_p[c, off:off + n] = norm[a:b]
            off += int(cpt[t]) * 128
    return src_p, dl_p, nm_p, cpt


def _prep_edges_balanced(edge_index, n_nodes, npc, ntiles, tpad):
    """p-net variant: per-core degree-balanced node->tile assignment."""
    npt = npc // ntiles                    # nodes per tile (125)
    src = np.asarray(edge_index[0], dtype=np.int64)
    dst = np.asarray(edge_index[1], dtype=np.int64)
    loops = np.arange(n_nodes, dtype=np.int64)
    src = np.concatenate([src, loops])
    dst = np.concatenate([dst, loops])
    deg = np.bincount(dst, minlength=n_nodes).astype(np.float32)
    dis = 1.0 / np.sqrt(deg)
    norm = (dis[src] * dis[dst]).astype(np.float32)

    tile_of = np.empty(n_nodes, np.int64)
    pos_of = np.empty(n_nodes, np.int64)
    for c in range(NC):
        lo = c * npc
        nodes = np.arange(lo, lo + npc)
        order = np.argsort(-deg[nodes], kind="stable")
        loads = np.zeros(ntiles)
        fill = np.zeros(ntiles, np.int64)
        for nd in nodes[order]:
            cand = np.where(fill < npt)[0]
            t = cand[np.argmin(loads[cand])]
            tile_of[nd] = t
            pos_of[nd] = fill[t]
            loads[t] += deg[nd]
            fill[t] += 1
    tid = tile_of[dst]
    dloc = pos_of[dst]
    key = (dst // npc) * ntiles + tid
    order = np.argsort(key, kind="stable")
    src, dloc, norm, key = src[order], dloc[order], norm[order], key[order]
    counts = np.bincount(key, minlength=NC * ntiles).reshape(NC, ntiles)
    cpt = np.maximum(1, -(-counts.max(axis=0) // 128)).astype(int)
    csum = np.concatenate([[0], np.cumsum(counts.ravel())])
    epc = int(cpt.sum()) * 128
    src_p = np.zeros((NC, epc), np.int64)
    dl_p = np.full((NC, epc), 200, np.int64)   # pads miss the 0..127 iota
    nm_p = np.zeros((NC, epc), np.float32)
    for c in range(NC):
        off = 0
        for t in range(ntiles):
            k = c * ntiles + t
            a, b = int(csum[k]), int(csum[k + 1])
            n = b - a
            src_p[c, off:off + n] = src[a:b]
            dl_p[c, off:off + n] = dloc[a:b]
            nm_p[c, off:off + n] = norm[a:b]
            off += int(cpt[t]) * 128
    perm = np.full((NC, tpad), -1, np.int64)
    for nd in range(n_nodes):
        c = nd // npc
        perm[c, tile_of[nd] * 128 + pos_of[nd]] = nd
    return src_p, dl_p, nm_p, cpt, perm, dis


def _pack_idx(src):
    """[NC, E] node ids -> [NC, 128, E//16] int16 (16-partition wrap,
    replicated to all 8 partition groups)."""
    n = src.shape[1]
    w = src.astype(np.int16).reshape(NC, n // 16, 16).transpose(0, 2, 1)
    return np.ascontiguousarray(np.tile(w, (1, 8, 1)))


def _pack_lane(vals, dtype):
    """[NC, E] per-edge values -> [NC, 128, E//128] lane-major tables."""
    n = vals.shape[1]
    w = vals.reshape(NC, n // 128, 128).transpose(0, 2, 1)
    return np.ascontiguousarray(w.astype(dtype))


def _build_G(src, nm, xT):
    """[NC, E] src ids + per-edge norm + [k, N] f32 ext features ->
    [NC, 128, nch, k] bf16 norm-folded pre-gathered chunks (lhsT)."""
    g = xT[:, src] * nm[None, :, :]              # [k, NC, E]
    g = np.transpose(g, (1, 2, 0))               # [NC, E, k]
    nch = g.shape[1] // 128
    k = g.shape[2]
    return np.ascontiguousarray(
        g.reshape(NC, nch, 128, k).transpose(0, 2, 1, 3).astype(NPBF))


# ------------------------------------------------------------- device build

def _build(cpt_p, cpt_v):
    nc = bacc.Bacc("TRN2", target_bir_lowering=False, debug=False,
                   num_devices=NC, num_swdge_queues=NQ)

    chp = int(cpt_p.sum())          # total p chunks per core
    chv = int(cpt_v.sum())
    cmax = int(cpt_p.max())
    coff_p = np.concatenate([[0], np.cumsum(cpt_p)]).astype(int)
    coff_v = np.concatenate([[0], np.cumsum(cpt_v)]).astype(int)
    pairs = [(t, t + 1) for t in range(0, NPT, 2)]

    def din(name, shape, dtype=F32):
        return nc.dram_tensor(name, shape, dtype, kind="ExternalInput")

    # inputs
    pxT_o = din("p_xT_own", [KP, PPAD])
    vxT_o = din("v_xT_own", [KV, NVC])
    wf1p = din("wf1p", [KP, D], BF16)
    wep = din("wep", [KP, D])
    w2p = din("w2p", [D, D], BF16)
    b1p = din("b1p", [D, 1])
    b2p = din("b2p", [D, 1])
    wf1v = din("wf1v", [KV, D], BF16)
    wev = din("wev", [KV, D])
    w2v = din("w2v", [D, D], BF16)
    b1v = din("b1v", [D, 1])
    b2v = din("b2v", [D, 1])
    w1a = din("w1a", [D, D], BF16)
    w1b = din("w1b", [D, D], BF16)
    b1a = din("b1a", [D, 1])
    b1b = din("b1b", [D, 1])
    w2a = din("w2a", [D, D], BF16)
    w2b = din("w2b", [D, D], BF16)
    b2m = din("b2m", [D, 1])
    w3 = din("w3", [D, 1], BF16)
    b3 = din("b3", [1, 1])
    sel = din("sel", [128, NVT * GPC])
    gt = din("gt", [GPC, PPAD])
    b2r = din("b2r", [GPC, D])
    gp_d = din("Gp", [128, chp, KP], BF16)
    gv_d = din("Gv", [128, chv, KV], BF16)
    idxp2 = din("idxs_p2", [128, chp * 8], I16)
    idxv1 = din("idxs_v1", [128, chv * 8], I16)
    dstp_d = din("dstp", [128, chp])
    dstv_d = din("dstv", [128, chv])
    nrmv_d = din("nrmv", [128, chv])
    diso_d = din("diso", [128, NPT])
    gtt_d = din("gtt", [128, NPT * GPC])
    iota_d = din("iota", [128, 128])

    out_d = nc.dram_tensor("out", [1, PPAD], F32, kind="ExternalOutput")

    # internal DRAM (bf16 exchange)
    ccip = nc.dram_tensor("ccip", [PPAD, D], BF16, kind="Internal")
    ccop = nc.dram_tensor("ccop", [NC * PPAD, D], BF16, kind="Internal",
                          addr_space="Shared")
    cciv = nc.dram_tensor("cciv", [NVC, D], BF16, kind="Internal")
    ccov = nc.dram_tensor("ccov", [NC * NVC, D], BF16, kind="Internal",
                          addr_space="Shared")

    with tile.TileContext(nc) as tc:
        with (
            tc.tile_pool(name="meta", bufs=1) as meta,
            tc.tile_pool(name="bigp", bufs=3) as bigp,
            tc.tile_pool(name="vsml", bufs=1) as vsml,
            tc.tile_pool(name="stp", bufs=3) as stp,
            tc.tile_pool(name="ohp", bufs=1) as ohp,
            tc.tile_pool(name="gat", bufs=1) as gat,
            tc.tile_pool(name="psA", bufs=3, space="PSUM") as psA,
            tc.tile_pool(name="psB", bufs=4, space="PSUM") as psB,
            tc.tile_pool(name="psC", bufs=1, space="PSUM") as psC,
        ):
            def load(dram, eng=None, tag=None):
                t = meta.tile(list(dram.shape), dram.dtype,
                              tag=tag or dram.name)
                (eng or nc.sync).dma_start(out=t[:], in_=dram[:])
                return t

            iota_sb = load(iota_d)
            dstp_sb = load(dstp_d)
            gp_sb = load(gp_d)
            wf1p_sb = load(wf1p)
            b1p_sb = load(b1p)
            wep_sb = load(wep)
            diso_sb = load(diso_d)
            idxp2_sb = load(idxp2, nc.scalar)
            idxv1_sb = load(idxv1, nc.scalar)
            dstv_sb = load(dstv_d, nc.scalar)
            nrmv_sb = load(nrmv_d, nc.scalar)
            gv_sb = load(gv_d, nc.scalar)
            gtt_sb = load(gtt_d, nc.scalar)
            b2r_sb = load(b2r, nc.scalar)
            w2p_sb = load(w2p, nc.scalar); b2p_sb = load(b2p, nc.scalar)
            wf1v_sb = load(wf1v, nc.scalar); wev_sb = load(wev, nc.scalar)
            w2v_sb = load(w2v, nc.scalar)
            b1v_sb = load(b1v, nc.scalar); b2v_sb = load(b2v, nc.scalar)
            w1a_sb = load(w1a, nc.scalar); w1b_sb = load(w1b, nc.scalar)
            b1a_sb = load(b1a, nc.scalar); b1b_sb = load(b1b, nc.scalar)
            w2a_sb = load(w2a, nc.scalar); w2b_sb = load(w2b, nc.scalar)
            b2m_sb = load(b2m, nc.scalar)
            w3_sb = load(w3, nc.scalar); b3_sb = load(b3, nc.scalar)
            sel_sb = load(sel, nc.scalar)
            gt_sb = load(gt, nc.scalar)

            ident = meta.tile([128, 128], F32, tag="ident")
            make_identity(nc, ident[:])

            initp = meta.tile([128, PPAD], BF16, tag="initp")
            initv = meta.tile([128, VPAD], F32, tag="initv")
            nc.vector.memset(initv[:], 0.0)

            pxTo_sb = meta.tile([KP, PPAD], F32, tag="pxTo")
            nc.sync.dma_start(out=pxTo_sb[:], in_=pxT_o[:])
            vxTo_sb = meta.tile([KV, NVC], F32, tag="vxTo")
            nc.sync.dma_start(out=vxTo_sb[:], in_=vxT_o[:])

            # ---- one-hot generation: one broadcast TT per dst tile
            def gen_oh01(ohb, c0, n_ch, dst_sb, eng):
                eng.tensor_tensor(
                    out=ohb[:, 0:n_ch, :],
                    in0=iota_sb[:].unsqueeze(1)
                        .broadcast_to([128, n_ch, 128]),
                    in1=dst_sb[:, c0:c0 + n_ch].unsqueeze(2)
                        .broadcast_to([128, n_ch, 128]),
                    op=OP.is_equal)

            # ---- initT own (feat x own nodes, tile layout)
            for j in range(PPAD // 512):
                acc = psB.tile([128, 512], F32, tag="b512", space="PSUM")
                nc.tensor.matmul(acc[:],
                                 wep_sb[:],
                                 pxTo_sb[:, j * 512:(j + 1) * 512],
                                 start=True, stop=True)
                nc.vector.tensor_copy(out=initp[:, j * 512:(j + 1) * 512],
                                      in_=acc[:])
            acc = psB.tile([128, 512], F32, tag="b512", space="PSUM")
            nc.tensor.matmul(acc[:, :NVC], wev_sb[:], vxTo_sb[:],
                             start=True, stop=True)
            nc.vector.tensor_copy(out=initv[:, :NVC], in_=acc[:, :NVC])

            # ---- layer 1 (p then v), xw2 fused per 4 tiles
            def l1_tile(t, g_sb, kd, cpt, coff, dst_sb, h_sb, b_sb,
                        wf_sb, eng):
                n_ch = int(cpt[t])
                ohb = ohp.tile([128, cmax, 128], BF16, tag=f"oh{t % 3}")
                gen_oh01(ohb, int(coff[t]), n_ch, dst_sb, eng)
                s_ps = psA.tile([128, 136], F32, tag="pacc", space="PSUM")
                for j in range(n_ch):
                    nc.tensor.matmul(
                        s_ps[:kd, :128],
                        g_sb[:, int(coff[t]) + j, :],
                        ohb[:, j, :],
                        start=(j == 0), stop=(j == n_ch - 1))
                s_sb = stp.tile([kd, 128], BF16, tag=f"ssb{kd}")
                nc.vector.tensor_copy(out=s_sb[:], in_=s_ps[:kd, :128])
                acc = psA.tile([128, 136], F32, tag="pacc", space="PSUM")
                nc.tensor.matmul(acc[:, :128], wf_sb[:], s_sb[:],
                                 start=True, stop=True)
                nc.scalar.activation(
                    out=h_sb[:, t * 128:(t + 1) * 128], in_=acc[:, :128],
                    func=mybir.ActivationFunctionType.Relu,
                    bias=b_sb[:, 0:1])

            def xw2_block(h_sb, w_sb, cci, off, nb, nrows, dis_sb):
                acc = psB.tile([128, nb * 128], F32, tag="b512",
                               space="PSUM")
                for j in range(nb):
                    i = off + j
                    nc.tensor.matmul(
                        acc[:, j * 128:(j + 1) * 128],
                        h_sb[:, i * 128:(i + 1) * 128],
                        w_sb[:], start=True, stop=True)
                stg = stp.tile([128, nb, 128], BF16, tag="stg")
                if dis_sb is None:
                    nc.vector.tensor_copy(out=stg[:], in_=acc[:])
                else:
                    for j in range(nb):
                        nc.vector.tensor_tensor(
                            out=stg[:, j, :],
                            in0=acc[:, j * 128:(j + 1) * 128],
                            in1=dis_sb[:, off + j:off + j + 1]
                                .broadcast_to([128, 128]),
                            op=OP.mult)
                nfull = min(nb, (nrows - off * 128) // 128)
                if nfull > 0:
                    nc.sync.dma_start(
                        out=cci[off * 128:(off + nfull) * 128, :]
                        .rearrange("(t p) f -> p t f", p=128),
                        in_=stg[:, :nfull, :])
                rem = nrows - (off + nfull) * 128
                if 0 < rem < 128 and nfull < nb:
                    nc.sync.dma_start(
                        out=cci[(off + nfull) * 128:nrows, :],
                        in_=stg[:rem, nfull, :])

            # ---- v layer 1 + exchange first: AG-v and the v gather
            #      hide entirely under p layer 1 / AG-p
            h1v = vsml.tile([128, VPAD], BF16, tag="h1v")
            for t in range(NVT):
                l1_tile(t, gv_sb, KV, cpt_v, coff_v, dstv_sb, h1v,
                        b1v_sb, wf1v_sb, nc.vector)
            xw2_block(h1v, w2v_sb, cciv, 0, 4, NVC, None)
            nc.gpsimd.collective_compute(
                "AllGather", OP.bypass,
                ins=[cciv[:]], outs=[ccov[:]],
                replica_groups=[list(range(NC))])
            vgb = gat.tile([128, chv, D], BF16, tag="gbv")
            nc.gpsimd.dma_gather(
                vgb[:], ccov[:], idxv1_sb[:], chv * 128, chv * 128, D,
                single_packet=False, queue_num=2)

            h1p = bigp.tile([128, PPAD], BF16, tag="bigh")
            for t in range(NPT):
                l1_tile(t, gp_sb, KP, cpt_p, coff_p, dstp_sb, h1p,
                        b1p_sb, wf1p_sb, nc.vector)
                if t % 4 == 3:
                    xw2_block(h1p, w2p_sb, ccip, t - 3, 4, PPAD, diso_sb)

            nc.gpsimd.collective_compute(
                "AllGather", OP.bypass,
                ins=[ccip[:]], outs=[ccop[:]],
                replica_groups=[list(range(NC))])

            # ---- p layer 2: direct gathers in pairs, scatter chasing
            h2p = bigp.tile([128, PPAD], BF16, tag="bigh")
            gsps = psC.tile([GPC, D], F32, tag="gs", space="PSUM")

            def scatter_ptile(t, m, mbase):
                n_ch = int(cpt_p[t])
                ohb = ohp.tile([128, cmax, 128], BF16, tag=f"oh{t % 3}")
                gen_oh01(ohb, int(coff_p[t]), n_ch, dstp_sb, nc.vector)
                acc = psA.tile([128, 136], F32, tag="pacc", space="PSUM")
                for j in range(n_ch):
                    nc.tensor.matmul(
                        acc[:, :128], ohb[:, j, :], m[:, mbase + j, :],
                        start=(j == 0), stop=(j == n_ch - 1))
                h2T = stp.tile([128, 128], F32, tag=f"h2T{t % 3}")
                nc.vector.tensor_tensor(
                    out=h2T[:], in0=acc[:, :128],
                    in1=diso_sb[:, t:t + 1].broadcast_to([128, 128]),
                    op=OP.mult)
                trp = psA.tile([128, 136], F32, tag="pacc", space="PSUM")
                nc.tensor.transpose(trp[:, :128], h2T[:], ident[:])
                nc.scalar.activation(
                    out=h2p[:, t * 128:(t + 1) * 128], in_=trp[:, :128],
                    func=mybir.ActivationFunctionType.Identity,
                    bias=b2p_sb[:, 0:1])
                nc.tensor.matmul(gsps[:], gtt_sb[:, t * GPC:(t + 1) * GPC],
                                 h2T[:], start=(t == 0), stop=(t == NPT - 1))

            dbufs = {}

            def direct_pair(i):
                t0, t1 = pairs[i]
                n_ch = int(cpt_p[t0] + cpt_p[t1])
                buf = gat.tile([128, 2 * cmax, D], BF16, tag=f"gd{i % 6}")
                dbufs[i] = buf
                e0 = int(coff_p[t0]) * 128
                n = n_ch * 128
                nc.gpsimd.dma_gather(
                    buf[:, :n_ch, :], ccop[:],
                    idxp2_sb[:, e0 // 16:(e0 + n) // 16],
                    n, n, D, single_packet=False, queue_num=i % NQ)

            direct_pair(0)
            direct_pair(1)
            direct_pair(2)
            h2v = vsml.tile([128, VPAD], F32, tag="h2v")
            for i in range(len(pairs)):
                if i + 3 < len(pairs):
                    direct_pair(i + 3)
                t0, t1 = pairs[i]
                buf = dbufs.pop(i)
                scatter_ptile(t0, buf, 0)
                scatter_ptile(t1, buf, int(cpt_p[t0]))
                if i == 0:
                    # v layer 2 (0/1 one-hot then per-lane norm fold)
                    for t in range(NVT):
                        n_ch = int(cpt_v[t])
                        ohb = ohp.tile([128, cmax, 128], BF16,
                                       tag=f"oh{t % 3}")
                        gen_oh01(ohb, int(coff_v[t]), n_ch, dstv_sb,
                                 nc.vector)
                        nc.vector.tensor_tensor(
                            out=ohb[:, 0:n_ch, :], in0=ohb[:, 0:n_ch, :],
                            in1=nrmv_sb[:, int(coff_v[t]):
                                        int(coff_v[t]) + n_ch]
                                .unsqueeze(2)
                                .broadcast_to([128, n_ch, 128]),
                            op=OP.mult)
                        acc = psA.tile([128, 136], F32, tag="pacc",
                                       space="PSUM")
                        for j in range(n_ch):
                            nc.tensor.matmul(
                                acc[:, :128],
                                vgb[:, int(coff_v[t]) + j, :],
                                ohb[:, j, :],
                                start=(j == 0), stop=(j == n_ch - 1))
                        nc.scalar.activation(
                            out=h2v[:, t * 128:(t + 1) * 128],
                            in_=acc[:, :128],
                            func=mybir.ActivationFunctionType.Identity,
                            bias=b2v_sb[:, 0:1])

            # ---- v pools / v summed / cur_v
            gv = vsml.tile([128, GPC], F32, tag="gv")
            for g in range(GPC):
                nc.vector.reduce_sum(out=gv[:, g:g + 1],
                                     in_=h2v[:, g * NVG:(g + 1) * NVG],
                                     axis=AX.X)
            gvs = vsml.tile([128, GPC], F32, tag="gvs")
            nc.vector.tensor_scalar(out=gvs[:], in0=gv[:],
                                    scalar1=1.0 / NVG, scalar2=None,
                                    op0=OP.mult)
            vsum = vsml.tile([128, VPAD], F32, tag="vsum")
            nc.vector.tensor_tensor(out=vsum[:], in0=h2v[:], in1=initv[:],
                                    op=OP.add)
            for g in range(GPC):
                nc.scalar.activation(
                    out=vsum[:, g * NVG:(g + 1) * NVG],
                    in_=vsum[:, g * NVG:(g + 1) * NVG],
                    func=mybir.ActivationFunctionType.Identity,
                    bias=gvs[:, g:g + 1])
            curv_ps = psA.tile([128, GPC], F32, tag="pacc", space="PSUM")
            for k in range(NVT):
                trp = psA.tile([128, 128], F32, tag="pacc", space="PSUM")
                nc.tensor.transpose(trp[:], vsum[:, k * 128:(k + 1) * 128],
                                    ident[:])
                vs_rm = vsml.tile([128, 128], F32, tag="vsrm")
                nc.vector.tensor_copy(out=vs_rm[:], in_=trp[:])
                nc.tensor.matmul(curv_ps[:], vs_rm[:],
                                 sel_sb[:, k * GPC:(k + 1) * GPC],
                                 start=(k == 0), stop=(k == NVT - 1))
            curv_sb = vsml.tile([128, GPC], F32, tag="curvsb")
            nc.vector.tensor_copy(out=curv_sb[:], in_=curv_ps[:])

            # gcT[g, d] = gsum/NPG + b2 + curv  (graph mean of h2 + cur_v)
            curvT_ps = psA.tile([128, 128], F32, tag="pacc", space="PSUM")
            nc.tensor.transpose(curvT_ps[:GPC, :], curv_sb[:], ident[:])
            gcT = vsml.tile([GPC, 128], F32, tag="gcT")
            nc.vector.tensor_scalar(out=gcT[:], in0=gsps[:],
                                    scalar1=1.0 / NPG, scalar2=None,
                                    op0=OP.mult)
            nc.vector.tensor_tensor(out=gcT[:], in0=gcT[:],
                                    in1=b2r_sb[:], op=OP.add)
            nc.vector.tensor_tensor(out=gcT[:], in0=gcT[:],
                                    in1=curvT_ps[:GPC, :], op=OP.add)

            state = bigp.tile([128, PPAD], BF16, tag="bigh")
            nc.vector.tensor_tensor(out=state[:], in0=h2p[:],
                                    in1=initp[:], op=OP.add)
            for n in range(PPAD // 512):
                sl = slice(n * 512, (n + 1) * 512)
                gcx = psB.tile([128, 512], F32, tag="b512", space="PSUM")
                nc.tensor.matmul(gcx[:], gcT[:], gt_sb[:, sl],
                                 start=True, stop=True)
                nc.vector.tensor_tensor(out=state[:, sl],
                                        in0=state[:, sl],
                                        in1=gcx[:], op=OP.add)

            # ---- MLP
            mh1a = bigp.tile([128, PPAD], BF16, tag="bigh")
            mh1b = bigp.tile([128, PPAD], BF16, tag="bigh")
            for (w_sb, b_sb, mh) in ((w1a_sb, b1a_sb, mh1a),
                                     (w1b_sb, b1b_sb, mh1b)):
                for n in range(PPAD // 512):
                    sl = slice(n * 512, (n + 1) * 512)
                    acc = psB.tile([128, 512], F32, tag="b512",
                                   space="PSUM")
                    nc.tensor.matmul(acc[:], w_sb[:], state[:, sl],
                                     start=True, stop=True)
                    nc.scalar.activation(
                        out=mh[:, sl], in_=acc[:],
                        func=mybir.ActivationFunctionType.Relu,
                        bias=b_sb[:, 0:1])
            mh2 = bigp.tile([128, PPAD], BF16, tag="bigh")
            for n in range(PPAD // 512):
                sl = slice(n * 512, (n + 1) * 512)
                acc = psB.tile([128, 512], F32, tag="b512", space="PSUM")
                nc.tensor.matmul(acc[:], w2a_sb[:], mh1a[:, sl],
                                 start=True, stop=False)
                nc.tensor.matmul(acc[:], w2b_sb[:], mh1b[:, sl],
                                 start=False, stop=True)
                nc.scalar.activation(
                    out=mh2[:, sl], in_=acc[:],
                    func=mybir.ActivationFunctionType.Relu,
                    bias=b2m_sb[:, 0:1])
            for n in range(PPAD // 512):
                sl = slice(n * 512, (n + 1) * 512)
                accl = psA.tile([1, 512], F32, tag="pacc", space="PSUM")
                nc.tensor.matmul(accl[:], w3_sb[:], mh2[:, sl],
                                 start=True, stop=True)
                lgc = stp.tile([1, 512], F32, tag="lgc")
                nc.vector.tensor_scalar(
                    out=lgc[0:1, :], in0=accl[:], scalar1=b3_sb[0:1, 0:1],
                    scalar2=None, op0=OP.add)
                nc.sync.dma_start(out=out_d[0:1, sl], in_=lgc[0:1, :])

    nc.compile()
    return nc


# ------------------------------------------------------------------- kernel

def kernel(**inputs):
    global LAST_EXEC_NS
    f = lambda k: np.asarray(inputs[k], dtype=np.float32)

    # edge preprocessing
    sp, dlp, nmp, cpt_p, perm, disp = _prep_edges_balanced(
        np.asarray(inputs["p_edge_index"]), NP, NPC, NPT, PPAD)
    sv, dlv, nmv, cpt_v = _prep_edges(np.asarray(inputs["v_edge_index"]),
                                      NV, NVC, NVT)
    # L2 p rows live at permuted positions: node -> core*PPAD + tile*128+pos
    posmap = np.empty(NP, np.int64)          # node -> tile*128+pos
    for c in range(NC):
        valid = perm[c] >= 0
        posmap[perm[c][valid]] = np.nonzero(valid)[0]
    sp2 = (sp // NPC) * PPAD + posmap[sp]

    idxs_p2 = _pack_idx(sp2)
    idxs_v1 = _pack_idx(sv)
    dstp = _pack_lane(dlp, np.float32)
    dstv = _pack_lane(dlv, np.float32)
    nrmv = _pack_lane(nmv, np.float32)
    iota = np.ascontiguousarray(
        np.broadcast_to(np.arange(128, dtype=np.float32)[None, :],
                        (128, 128)))

    # weights
    p_x = f("p_x"); v_x = f("v_x")
    wep = np.vstack([f("p_lin_w"), f("p_lin_b")[None, :]])
    wev = np.vstack([f("v_lin_w"), f("v_lin_b")[None, :]])
    wf1p = wep @ f("p_gcn_w1")
    wf1v = wev @ f("v_gcn_w1")
    pxT = np.vstack([p_x.T, np.ones((1, NP), np.float32)])
    vxT = np.vstack([v_x.T, np.ones((1, NV), np.float32)])
    act = np.asarray(inputs["high_level_action"]).astype(np.int64)

    # pre-gathered per-edge features with edge norm folded in
    Gp = _build_G(sp, nmp, pxT)
    Gv = _build_G(sv, nmv, vxT)

    base = {
        "wf1p": wf1p.astype(NPBF), "wep": wep,
        "w2p": f("p_gcn_w2").astype(NPBF),
        "b1p": f("p_gcn_b1")[:, None], "b2p": f("p_gcn_b2")[:, None],
        "wf1v": wf1v.astype(NPBF), "wev": wev,
        "w2v": f("v_gcn_w2").astype(NPBF),
        "b1v": f("v_gcn_b1")[:, None], "b2v": f("v_gcn_b2")[:, None],
        "w1a": f("low_w1")[:, :D].astype(NPBF),
        "w1b": f("low_w1")[:, D:].astype(NPBF),
        "b1a": f("low_b1")[:D, None], "b1b": f("low_b1")[D:, None],
        "w2a": f("low_w2")[:D, :].astype(NPBF),
        "w2b": f("low_w2")[D:, :].astype(NPBF),
        "b2m": f("low_b2")[:, None],
        "w3": f("low_w3").astype(NPBF), "b3": f("low_b3")[:, None],
        "b2r": np.broadcast_to(f("p_gcn_b2")[None, :], (GPC, D)),
        "iota": iota,
    }
    base = {k: (np.ascontiguousarray(v) if v.dtype == NPBF
                else np.ascontiguousarray(v, dtype=np.float32))
            for k, v in base.items()}

    pgraph = np.asarray(inputs["p_batch"]).astype(np.int64)

    in_maps = []
    for c in range(NC):
        selm = np.zeros((128, NVT * GPC), np.float32)
        for g in range(GPC):
            r = g * NVG + int(act[c * GPC + g])
            selm[r % 128, (r // 128) * GPC + g] = 1.0
        pxo = np.zeros((KP, PPAD), np.float32)
        gtm = np.zeros((GPC, PPAD), np.float32)
        disown = np.zeros(PPAD, np.float32)
        gttm = np.zeros((128, NPT * GPC), np.float32)
        valid = perm[c] >= 0
        cols = np.nonzero(valid)[0]
        nodes = perm[c][valid]
        pxo[:, cols] = pxT[:, nodes]
        gtm[pgraph[nodes] % GPC, cols] = 1.0
        disown[cols] = disp[nodes]
        gttm[cols % 128, (cols // 128) * GPC + pgraph[nodes] % GPC] = 1.0
        m = dict(base)
        m["p_xT_own"] = pxo
        m["v_xT_own"] = np.ascontiguousarray(
            vxT[:, c * NVC:(c + 1) * NVC])
        m["sel"] = selm
        m["gt"] = gtm
        m["diso"] = np.ascontiguousarray(
            disown.reshape(NPT, 128).T.astype(np.float32))
        m["gtt"] = np.ascontiguousarray(gttm)
        m["idxs_p2"] = idxs_p2[c]
        m["idxs_v1"] = idxs_v1[c]
        m["dstp"] = dstp[c]; m["dstv"] = dstv[c]; m["nrmv"] = nrmv[c]
        m["Gp"] = Gp[c]; m["Gv"] = Gv[c]
        in_maps.append(m)

    nc = _build(cpt_p, cpt_v)
    res = run_bass_kernel_spmd(nc, in_maps, core_ids=list(range(NC)),
                               trace=TRACE)
    LAST_EXEC_NS = res.exec_time_ns
    out = np.empty((NC, NPC), np.float32)
    for c in range(NC):
        lgv = res.results[c]["out"][0]
        valid = perm[c] >= 0
        out[c][perm[c][valid] - c * NPC] = lgv[valid]
    return out.reshape(B, NPG).astype(np.float32)
